# revision 43
# baseline (speedup 1.0000x reference)
# Trainium2 Bass kernel for residual-VQ autoencoder loss (vq_codebook).
# Data-parallel over rows: 8 NeuronCores, 2048 rows each; codebooks/weights
# replicated. The scalar loss is assembled on host from per-core [128,64]
# partial-sum outputs.
#
# fp8 (e4m3) DoubleRow matmuls everywhere it pays: enc layer 1, all four
# VQ score levels, and both decoder layers run with 2 k-tiles per PE pass
# at 0.5 cyc/col (4x bf16 FLOP rate). VQ scores are PURE dot products:
# the -|E|^2 argmin correction is handled by sorting each level's vocab by
# |E|^2 into the 4 scan groups (host-side permutation, gather-table agrees)
# and adding a grid-quantized per-group bias during index extraction.
# Scores may be negative (no SHIFT); the QPACK2 pack/extract math is
# two's-complement safe.
#
# Per core (RT=16 row-tiles of 128):
#   encoder: h = x8@W1_8 (fp8) -> LN -> ReLU(bf16) -> latent = h@W2' (bf16,
#   ln_g folded into W2).
#   RVQ pair-max argmax per level:
#     psE/psO = fp8 DoubleRow matmuls (1024 cols each, 4 groups/row-tile);
#     ACT copies psO -> SBUF; QPACK2 (custom DVE) packs
#     quant2048(max(odd,even)) + 2*i + (odd>=even), MAX-accum per group.
#     Extraction adds the per-group |E|^2 bias, picks the group, recovers
#     the vocab index; gpsimd ap_gather fetches exact bf16 codewords;
#     resid -= q^T.  residT8 (fp8, scale K_r[lv]) feeds the next level.
#   loss telescopes: sum_l 1.5*mean((q_l-r_l)^2) = 1.5*(R0-R4)/(N*LAT);
#   decoder: quant8 = K_q*(latT-residT) -> fp8 DoubleRow MLP; Rrec uses
#   K_rec-scaled x^T from DRAM; host divides by K_rec^2.

import sys

sys.path.insert(0, "/opt/trn_rl_repo")

import numpy as np

import concourse.bass as bass
import concourse.mybir as mybir
import concourse.bacc as bacc
import concourse.tile as tile
from concourse.alu_op_type import AluOpType
from concourse.bass_utils import run_bass_kernel_spmd

OBS, HID, LAT = 1024, 2048, 256
VOCAB, HQ = 8192, 4
N, NCORES = 16384, 8
NSH = N // NCORES          # 2048 rows per core
RT = NSH // 128            # 16 row tiles
LN_EPS = 1e-5
GRID = 2048.0               # 11 bits: 2*i + b per 1024-pair scan group
BIG = float(1.5 * 2.0**34)  # ulp(BIG)=2048 -> (x+BIG)-BIG rounds to mult of GRID

f32 = mybir.dt.float32
bf16 = mybir.dt.bfloat16
fp8 = mybir.dt.float8e4
i16 = mybir.dt.int16

_QPACK2 = None


def _register_qpack2():
    """packed = quant2048(max(Src0,Src1)) + 2*idx + (Src0>=Src1);
    accum_out = max over free dim.  Src0: odd scores (SBUF f32, ACT-copied
    from PSUM); Src1: even scores (PSUM f32).  With Src0=odds, the local
    vocab index within the 2048-wide group is exactly packed mod 2048."""
    global _QPACK2
    if _QPACK2 is not None:
        return _QPACK2
    from concourse import dve_ops
    from concourse.dve_spec import (
        Spec, Src0, Src1, C1, C2, AluOp, lower, Bin, Zero, Scan,
    )
    from concourse.dve_table_gen import DveOpSpec

    def _ref(in0, in1, s0, s1, imm2):
        o = np.asarray(in0, np.float32)
        e = np.asarray(in1, np.float32)
        m = np.maximum(o, e)
        q = (m + np.float32(imm2)).astype(np.float32) - np.float32(imm2)
        idx2 = np.arange(o.shape[-1], dtype=np.float32) * np.float32(s1)
        out = ((q + idx2) + (o >= e).astype(np.float32)).astype(np.float32)
        acc = out.max(axis=-1, keepdims=True)
        return out, acc

    m = Bin(AluOp.MAX, Src0, Src1)
    q = Bin(AluOp.SUBTRACT, Bin(AluOp.ADD, m, C2), C2)
    idx2 = Scan(AluOp.ADD, C1, init=Bin(AluOp.SUBTRACT, Zero, C1))
    g = Bin(AluOp.IS_GE, Src0, Src1)
    body = Bin(AluOp.ADD, Bin(AluOp.ADD, q, idx2), g)
    spec = Spec(body=body, accum=AluOp.MAX, reference=_ref)
    op = dve_ops.DveOp("QPACK2_PAIRMAX", spec, subdim=False, uops_sha={})
    dve_ops.OPS.append(op)
    dve_ops.CUSTOM_DVE_SPECS[op.name] = op.spec
    dve_ops._SUB_OPCODE_FOR_NAME[op.name] = (
        dve_ops._CUSTOM_DVE_ROW_BASE + len(dve_ops.OPS) - 1
    )
    for ver in ("v3", "v4"):
        s = DveOpSpec(
            name=op.name,
            opcode=dve_ops.get_dve_sub_opcode(op.name),
            uops=lower(spec, ver=ver),
            rd1_en=True,
        )
        op.uops_sha[ver] = s.sha(ver)
    _QPACK2 = op
    return op


def build_nc(krs, kq, drelu):
    """krs: per-level resid fp8 scales (len HQ); kq: quant fp8 scale;
    drelu: decoder hidden ACT scale S_dh/(K_q*S_d1)."""
    qpack2 = _register_qpack2()
    nc = bacc.Bacc(
        "TRN2",
        target_bir_lowering=False,
        debug=False,
        enable_asserts=False,
        num_devices=NCORES,
    )
    Relu = mybir.ActivationFunctionType.Relu
    Square = mybir.ActivationFunctionType.Square
    Sqrt = mybir.ActivationFunctionType.Sqrt
    Copy = mybir.ActivationFunctionType.Copy
    DR = mybir.MatmulPerfMode.DoubleRow

    # ---------------- DRAM I/O ----------------
    x8t_d = nc.dram_tensor("x8t", [OBS, NSH], fp8, kind="ExternalInput")
    xtd_d = nc.dram_tensor("xtd", [OBS, NSH], bf16, kind="ExternalInput")
    w1_d = nc.dram_tensor("w1x8", [OBS, HID], fp8, kind="ExternalInput")
    w2_d = nc.dram_tensor("w2b", [HID, LAT], bf16, kind="ExternalInput")
    dw1_d = nc.dram_tensor("dw1x8", [LAT, HID], fp8, kind="ExternalInput")
    dw2_d = nc.dram_tensor("dw2x8", [HID, OBS], fp8, kind="ExternalInput")
    # even/odd score codebooks: [HQ, k, 128, VOCAB//2], |E|^2-sorted columns
    e2e_d = nc.dram_tensor("e2e8", [HQ, 2, 128, VOCAB // 2], fp8, kind="ExternalInput")
    e2o_d = nc.dram_tensor("e2o8", [HQ, 2, 128, VOCAB // 2], fp8, kind="ExternalInput")
    e2tp_d = nc.dram_tensor("e2tp", [HQ, 128, VOCAB], mybir.dt.uint32, kind="ExternalInput")
    d_giota = nc.dram_tensor("giota", [128, RT * 4], f32, kind="ExternalInput")
    d_gbias = nc.dram_tensor("gbias", [128, HQ * 4], f32, kind="ExternalInput")
    out_d = nc.dram_tensor("out", [128, 64], f32, kind="ExternalOutput")

    import contextlib

    with tile.TileContext(nc) as tc, contextlib.ExitStack() as ctx:
        const_p = ctx.enter_context(tc.tile_pool(name="const", bufs=1))
        persist_p = ctx.enter_context(tc.tile_pool(name="persist", bufs=1))
        small_p = ctx.enter_context(tc.tile_pool(name="small", bufs=4))

        # ---- constants ----
        out_sb = const_p.tile([128, 64], f32, name="out_sb")
        nc.vector.memset(out_sb[:], 0.0)
        epsc = const_p.tile([128, 1], f32, name="epsc")
        nc.vector.memset(epsc[:], LN_EPS)
        giota = const_p.tile([128, RT * 4], f32, name="giota")
        nc.sync.dma_start(giota[:], d_giota.ap())
        gbias = const_p.tile([128, HQ * 4], f32, name="gbias")
        nc.sync.dma_start(gbias[:], d_gbias.ap())

        # ---- persistent ----
        latT = persist_p.tile([128, 2, NSH], f32, name="latT")
        residT = persist_p.tile([128, 2, NSH], f32, name="residT")
        residT8 = persist_p.tile([128, 2, NSH], fp8, name="residT8")
        trash = persist_p.tile([128, 2048], bf16, name="trash")
        trash2 = persist_p.tile([128, 2048], bf16, name="trash2")
        dw1s = persist_p.tile([128, 2, HID], fp8, name="dw1s")
        dw2s = persist_p.tile([128, HID // 128, OBS], fp8, name="dw2s")
        qp = persist_p.tile([128, NSH], mybir.dt.uint32, name="qp")
        idx16 = persist_p.tile([128, RT], i16, name="idx16")
        idxg = persist_p.tile([128, RT, 8], i16, name="idxg")
        nc.vector.memset(idxg[:], 0)
        # =============== encoder ===============
        enc_ctx = contextlib.ExitStack()
        enc_p = enc_ctx.enter_context(tc.tile_pool(name="encp", bufs=1))
        work_p = enc_ctx.enter_context(tc.tile_pool(name="encw", bufs=3))
        eps_h = enc_ctx.enter_context(tc.tile_pool(name="epsh", bufs=6, space="PSUM"))
        w2s = enc_p.tile([128, HID // 128, LAT], bf16, name="w2s")
        w1s = enc_p.tile([128, OBS // 128, HID], fp8, name="w1s")
        xT_sb = enc_p.tile([128, OBS // 128, NSH], fp8, name="xT_sb")
        # order: x^T first-half columns + w1 cc0 (enough for rt0-7 cc0), then
        # the rest, so the first h-matmul group starts as early as possible
        for k in range(OBS // 128):
            nc.sync.dma_start(
                xT_sb[:, k, 0:1024], x8t_d.ap()[k * 128:(k + 1) * 128, 0:1024]
            )
        for k in range(OBS // 128):
            nc.sync.dma_start(
                w1s[:, k, 0:512], w1_d.ap()[k * 128:(k + 1) * 128, 0:512]
            )
        for k in range(OBS // 128):
            nc.sync.dma_start(
                xT_sb[:, k, 1024:2048], x8t_d.ap()[k * 128:(k + 1) * 128, 1024:2048]
            )
        for cc in range(1, 4):
            for k in range(OBS // 128):
                nc.sync.dma_start(
                    w1s[:, k, cc * 512:(cc + 1) * 512],
                    w1_d.ap()[k * 128:(k + 1) * 128, cc * 512:(cc + 1) * 512],
                )
        for k in range(HID // 128):
            nc.sync.dma_start(w2s[:, k, :], w2_d.ap()[k * 128:(k + 1) * 128, :])
        for k in range(2):
            nc.sync.dma_start(dw1s[:, k, :], dw1_d.ap()[k * 128:(k + 1) * 128, :])

        def enc_lat(rc, hTc):
            # latent^T for chunk rc's 256 rows (issued one chunk late so the
            # PE never head-of-line blocks on the h^T DMA transposes)
            for m in range(2):
                lps = eps_h.tile([128, 256], f32, name="lps", tag="hps")
                nk = HID // 128
                for k in range(nk):
                    nc.tensor.matmul(
                        lps[:, 0:256],
                        w2s[:, k, m * 128:(m + 1) * 128],
                        hTc[:, :, k, :],
                        start=(k == 0),
                        stop=(k == nk - 1),
                    )
                nc.vector.tensor_copy(
                    latT[:, m, rc * 256:(rc + 1) * 256], lps[:, 0:256]
                )

        prev = None  # (rc, hTc) of the previous chunk
        for rc in range(8):  # row chunks of 256 (2 row tiles each)
            # [p, rj, o, r]: rj-major so each transpose dest is contiguous
            hTc = work_p.tile([128, 2, HID // 128, 128], bf16, name="hTc", tag="hT")
            for rj in range(2):
                rt = rc * 2 + rj
                # h = x8 @ W1_8, fp8 DoubleRow, chunk-grained PSUM for overlap
                hsb = work_p.tile([128, 2048], f32, name="hsb", tag="hsb")
                bns = small_p.tile([128, 4, 6], f32, name="bns", tag="s1")
                for cc in range(4):
                    hps = eps_h.tile([128, 512], f32, name="hps", tag="hps")
                    for j in range(OBS // 256):
                        nc.tensor.matmul(
                            hps[:],
                            xT_sb[:, 2 * j:2 * j + 2, rt * 128:(rt + 1) * 128],
                            w1s[:, 2 * j:2 * j + 2, cc * 512:(cc + 1) * 512],
                            start=(j == 0),
                            stop=(j == OBS // 256 - 1),
                            perf_mode=DR,
                        )
                    nc.scalar.activation(
                        hsb[:, cc * 512:(cc + 1) * 512], hps[:],
                        mybir.ActivationFunctionType.Copy,
                    )
                    nc.vector.bn_stats(bns[:, cc, :], hps[:])
                mv = small_p.tile([128, 2], f32, name="mv", tag="s2")
                nc.vector.bn_aggr(mv[:], bns[:].rearrange("p a b -> p (a b)"))
                std = small_p.tile([128, 1], f32, name="std", tag="s6")
                nc.scalar.activation(std[:], mv[:, 1:2], Sqrt, bias=epsc[:])
                rstd = small_p.tile([128, 1], f32, name="rstd", tag="s7")
                nc.vector.reciprocal(rstd[:], std[:])
                nmr = small_p.tile([128, 1], f32, name="nmr", tag="s8")
                nc.vector.tensor_scalar(
                    nmr[:], mv[:, 0:1], rstd[:], -1.0,
                    op0=AluOpType.mult, op1=AluOpType.mult,
                )
                hrelu = work_p.tile([128, 2048], bf16, name="hrelu", tag="hrelu")
                nc.scalar.activation(
                    hrelu[:], hsb[:], Relu, bias=nmr[:], scale=rstd[:]
                )
                # h^T for this row tile via the DMA crossbar transpose
                nc.sync.dma_start_transpose(hTc[:, rj], hrelu[:])
            if prev is not None:
                enc_lat(*prev)
            prev = (rc, hTc)
        enc_lat(*prev)

        nc.vector.tensor_copy(residT[:, 0, :], latT[:, 0, :])
        nc.vector.tensor_copy(residT[:, 1, :], latT[:, 1, :])
        nc.scalar.activation(residT8[:, 0, :], latT[:, 0, :], Copy, scale=krs[0])
        nc.scalar.activation(residT8[:, 1, :], latT[:, 1, :], Copy, scale=krs[0])

        enc_ctx.close()

        # =============== RVQ ===============
        vq_ctx = contextlib.ExitStack()
        vq_p = vq_ctx.enter_context(tc.tile_pool(name="vqp", bufs=1))
        vps_p = vq_ctx.enter_context(tc.tile_pool(name="vps", bufs=2, space="PSUM"))
        sod_p = vq_ctx.enter_context(tc.tile_pool(name="sod", bufs=3))
        # [p, buf, k, c]: double-buffered even/odd codebooks
        e2es = vq_p.tile([128, 2, 2, VOCAB // 2], fp8, name="e2es")
        e2os = vq_p.tile([128, 2, 2, VOCAB // 2], fp8, name="e2os")
        e2tp = vq_p.tile([128, 2, VOCAB], mybir.dt.uint32, name="e2tp")
        from concourse import library_config
        nc.gpsimd.load_library(library_config.ap_gather)
        pk64 = persist_p.tile([128, RT * 4], f32, name="pk64")

        def load_level(lv):
            db = lv % 2
            for k in range(2):
                nc.sync.dma_start(e2es[:, db, k, :], e2e_d.ap()[lv, k])
                nc.sync.dma_start(e2os[:, db, k, :], e2o_d.ap()[lv, k])
            nc.sync.dma_start(e2tp[:, db, :], e2tp_d.ap()[lv])

        qb = qp[:].bitcast(bf16).rearrange("p (n two) -> p n two", two=2)

        def extraction_ops(lv, qq):
            """Index extraction + gather staging for quarter (lv, qq), as a
            list of closures (issued interleaved between QPACK2 groups)."""
            cs = qq * 16
            db = lv % 2
            m16 = small_p.tile([128, 4], f32, name="m16", tag="m16")
            pkb = small_p.tile([128, 4, 4], f32, name="pkb", tag="pkb")
            msk = small_p.tile([128, 4, 4], f32, name="msk", tag="msk")
            gidx = small_p.tile([128, 4], f32, name="gidx", tag="gidx")
            mi = small_p.tile([128, 4], mybir.dt.int32, name="mi", tag="mi")
            loc = small_p.tile([128, 4], f32, name="loc", tag="loc")
            pk3 = pk64[:, cs:cs + 16].rearrange("p (a b) -> p a b", a=4)
            gb3 = (gbias[:, lv * 4:(lv + 1) * 4]
                   .rearrange("p (o g) -> p o g", o=1)
                   .broadcast_to((128, 4, 4)))

            def s1():
                # add the per-group -K*mean|E|^2 correction (grid-quantized,
                # so the low 11 index bits survive)
                nc.vector.tensor_tensor(pkb[:], pk3, gb3, op=AluOpType.add)
                nc.vector.tensor_reduce(
                    m16[:], pkb[:], axis=mybir.AxisListType.X, op=AluOpType.max
                )
                nc.vector.tensor_tensor(
                    msk[:], pkb[:],
                    m16[:].rearrange("p (a o) -> p a o", o=1)
                    .broadcast_to((128, 4, 4)),
                    op=AluOpType.is_ge,
                )

            def s2():
                nc.vector.tensor_mul(
                    msk[:], msk[:],
                    giota[:, cs:cs + 16].rearrange("p (a b) -> p a b", a=4),
                )
                nc.vector.tensor_reduce(
                    gidx[:], msk[:], axis=mybir.AxisListType.X, op=AluOpType.add
                )
                nc.vector.tensor_scalar_min(gidx[:], gidx[:], 3.0)
                nc.vector.tensor_copy(mi[:], m16[:])
                nc.vector.tensor_scalar(
                    mi[:], mi[:], int(GRID) - 1, None, op0=AluOpType.bitwise_and
                )

            def s3():
                nc.vector.tensor_copy(loc[:], mi[:])
                nc.vector.tensor_scalar(
                    gidx[:], gidx[:], GRID, None, op0=AluOpType.mult
                )
                nc.vector.tensor_add(loc[:], loc[:], gidx[:])
                nc.vector.tensor_copy(idx16[:, qq * 4:(qq + 1) * 4], loc[:])

            def s4():
                # stage indices (wrapped + replicated across 8 Q7 groups).
                # The 8 wraps ride the idle SP HWDGE queue — the gpsimd SWDGE
                # descriptor-gen at ~0.6us each was saturating the Q7 queue
                # and pushing the gather (and everything waiting on it) late.
                for kk in range(8):
                    nc.sync.dma_start(
                        idxg[0:16, qq * 4:(qq + 1) * 4, kk],
                        idx16[kk * 16:(kk + 1) * 16, qq * 4:(qq + 1) * 4],
                    )
                # replicates on the SP HWDGE ring: the gpsimd SWDGE ring has
                # ~15us gen->complete latency (measured), which paced the
                # whole gather chain; SP completes small transfers in ~2us
                for gg in range(1, 8):
                    nc.sync.dma_start(
                        idxg[gg * 16:(gg + 1) * 16, qq * 4:(qq + 1) * 4, :],
                        idxg[0:16, qq * 4:(qq + 1) * 4, :],
                    )
                nc.gpsimd.ap_gather(
                    qp[:, qq * 512:(qq + 1) * 512],
                    e2tp[:, db, :],
                    idxg[:, qq * 4:(qq + 1) * 4, :].rearrange("p a b -> p (a b)"),
                    channels=128, num_elems=VOCAB, d=1, num_idxs=512,
                )

            return [s1, s2, s3, s4]

        def sub_ops(lv, qq):
            """resid -= q for quarter (lv, qq) on GPSIMD. With staging on
            the SP ring, the Q7 queue holds only compute ops (gathers+subs,
            no DMA<->compute DRAINs) and is ~95% idle, so the positional
            wait on the latest gather lands there instead of blocking the
            saturated DVE queue. Last level also computes quant = latT -
            residT (decoder input)."""
            c0, c1 = qq * 512, (qq + 1) * 512

            def u(m):
                def f():
                    nc.gpsimd.tensor_sub(
                        residT[:, m, c0:c1].rearrange("p (n o) -> p n o", o=1),
                        residT[:, m, c0:c1].rearrange("p (n o) -> p n o", o=1),
                        qb[:, c0:c1, m:m + 1],
                    )
                    if lv == HQ - 1:
                        nc.gpsimd.tensor_sub(
                            latT[:, m, c0:c1], latT[:, m, c0:c1],
                            residT[:, m, c0:c1],
                        )
                return f

            return [u(0), u(1)]

        def fin_ops(lv, qq):
            """fp8 residT8 refresh + loss square for quarter (lv, qq), issued
            a quarter after the gpsimd subs so the ACT queue never waits."""
            c0, c1 = qq * 512, (qq + 1) * 512

            def u(m):
                def f():
                    if lv < HQ - 1:
                        nc.scalar.activation(
                            residT8[:, m, c0:c1], residT[:, m, c0:c1],
                            Copy, scale=krs[lv + 1],
                        )
                    else:
                        nc.scalar.activation(
                            residT8[:, m, c0:c1], latT[:, m, c0:c1],
                            Copy, scale=kq,
                        )
                    # per-quarter loss accumulation (col lv*8 + m*4 + qq)
                    nc.scalar.activation(
                        trash2[:, 0:512], residT[:, m, c0:c1], Square,
                        accum_out=out_sb[:, lv * 8 + m * 4 + qq:
                                         lv * 8 + m * 4 + qq + 1],
                    )
                return f

            return [u(0), u(1)]

        def quarter_groups(lv, qq, chunks):
            """Issue the 16 matmul/QPACK2 groups of quarter (lv, qq), with the
            deferred closures of older quarters spread between row tiles."""
            db = lv % 2
            for rj in range(4):
                rt = qq * 4 + rj
                for g in range(4):
                    psE = vps_p.tile([128, 1024], f32, name="psE", tag="psE")
                    psO = vps_p.tile([128, 1024], f32, name="psO", tag="psO")
                    sodd = sod_p.tile([128, 1024], f32, name="sodd", tag="sodd")
                    c0 = g * 1024
                    for h in range(2):
                        nc.tensor.matmul(
                            psO[:, h * 512:(h + 1) * 512],
                            residT8[:, :, rt * 128:(rt + 1) * 128],
                            e2os[:, db, :, c0 + h * 512:c0 + (h + 1) * 512],
                            start=True, stop=True, perf_mode=DR,
                        )
                    nc.scalar.activation(sodd[:], psO[:], Copy)
                    for h in range(2):
                        nc.tensor.matmul(
                            psE[:, h * 512:(h + 1) * 512],
                            residT8[:, :, rt * 128:(rt + 1) * 128],
                            e2es[:, db, :, c0 + h * 512:c0 + (h + 1) * 512],
                            start=True, stop=True, perf_mode=DR,
                        )
                    nc.vector._custom_dve(
                        qpack2,
                        out=trash[:, 0:1024],
                        in0=sodd[:],
                        in1=psE[:],
                        s1=2.0,
                        imm2=BIG,
                        accum_out=pk64[:, rt * 4 + g: rt * 4 + g + 1],
                    )
                for f in chunks[rj]:
                    f()

        load_level(0)
        NQ = HQ * 4

        def hi(f):
            def g():
                with tc.high_priority(offset=600):
                    f()
            return g

        for Q in range(NQ + 1):
            lv, qq = divmod(Q, 4)
            # prefetch at qq==2: late enough that the staging DMAs of the
            # previous quarter (SP queue) never sit behind a 4MB codebook
            # transfer, early enough to land a level ahead of use
            if Q < NQ and qq == 2 and lv + 1 < HQ:
                load_level(lv + 1)
            if Q == 10:
                # prefetch decoder weights mid-VQ (DMA queue is idle here)
                for k in range(HID // 128):
                    nc.sync.dma_start(
                        dw2s[:, k, :], dw2_d.ap()[k * 128:(k + 1) * 128, :]
                    )
            # chunk schedule: extraction + gather staging of Q-1 at rj0;
            # subs (DVE) + finalize (ACT) of Q-3 spread over rj1-rj3 —
            # 2.5 quarters after that quarter's gather chain started, so the
            # in-order engine queues never idle waiting on it.
            chunks = [[], [], [], []]
            if 1 <= Q <= NQ:
                l1, q1 = divmod(Q - 1, 4)
                s1, s2, s3, s4 = extraction_ops(l1, q1)
                chunks[0] += [hi(s1), hi(s2), hi(s3), hi(s4)]
            if Q >= 3:
                l2, q2 = divmod(Q - 3, 4)
                u0, u1 = sub_ops(l2, q2)
                f0, f1 = fin_ops(l2, q2)
                chunks[1] += [u0]
                chunks[2] += [u1, f0]
                chunks[3] += [f1]
            if Q < NQ:
                quarter_groups(lv, qq, chunks)
            else:
                for c in chunks:
                    for f in c:
                        f()

        vq_ctx.close()

        # =============== decoder ===============
        # interleaved with the VQ epilogue: chunk rc only needs residT8
        # columns of quarter rc, so rc=0 runs while (3,3)'s gather/update
        # chain (issued right after) drains on DVE/ACT/GpSimd.
        dec_ctx = contextlib.ExitStack()
        work_p = dec_ctx.enter_context(tc.tile_pool(name="decw", bufs=2))
        dps_p = dec_ctx.enter_context(tc.tile_pool(name="dps", bufs=4, space="PSUM"))

        def dec_chunk(rc):
            dhT = work_p.tile([128, HID // 128, 512], fp8, name="dhT", tag="hT")
            for ht in range(HID // 128):
                dps = dps_p.tile([128, 512], f32, name="dps", tag="dmm")
                nc.tensor.matmul(
                    dps[:, 0:512],
                    dw1s[:, :, ht * 128:(ht + 1) * 128],
                    residT8[:, :, rc * 512:(rc + 1) * 512],
                    start=True, stop=True, perf_mode=DR,
                )
                nc.scalar.activation(dhT[:, ht, :], dps[:, 0:512], Relu, scale=drelu)
            for ot in range(OBS // 128):
                xTl = work_p.tile([128, 512], bf16, name="xTl", tag="xTl")
                nc.sync.dma_start(
                    xTl[:],
                    xtd_d.ap()[ot * 128:(ot + 1) * 128, rc * 512:(rc + 1) * 512],
                )
                rps = dps_p.tile([128, 512], f32, name="rps", tag="dmm")
                nk = HID // 256
                for k in range(nk):
                    nc.tensor.matmul(
                        rps[:, 0:512],
                        dw2s[:, 2 * k:2 * k + 2, ot * 128:(ot + 1) * 128],
                        dhT[:, 2 * k:2 * k + 2, :],
                        start=(k == 0), stop=(k == nk - 1), perf_mode=DR,
                    )
                diff = work_p.tile([128, 512], f32, name="diff", tag="diff")
                nc.vector.tensor_sub(diff[:], rps[:, 0:512], xTl[:])
                nc.scalar.activation(
                    diff[:], diff[:], Square,
                    accum_out=out_sb[:, 32 + rc * 8 + ot: 33 + rc * 8 + ot],
                )

        # (3,2) and (3,3) subs + finalize are still pending (defer-3);
        # interleave with the decoder chunks so their chains drain under
        # dec PE work
        dec_chunk(0)
        for f in sub_ops(3, 2):
            hi(f)()
        dec_chunk(1)
        for f in fin_ops(3, 2):
            hi(f)()
        for f in sub_ops(3, 3):
            hi(f)()
        dec_chunk(2)
        for f in fin_ops(3, 3):
            hi(f)()
        dec_chunk(3)

        dec_ctx.close()
        nc.sync.dma_start(out_d.ap(), out_sb[:])

    nc.compile()
    return nc


def _host_prep(inputs):
    import ml_dtypes

    f8 = ml_dtypes.float8_e4m3

    def q8c(v):
        return np.clip(v, -224.0, 224.0).astype(f8)

    x = np.asarray(inputs["x"], np.float32)
    cb = np.ascontiguousarray(np.asarray(inputs["codebooks"], np.float32))
    w1 = np.ascontiguousarray(np.asarray(inputs["enc_w1"], np.float32))
    b1 = np.asarray(inputs["enc_b1"], np.float32)
    lng = np.asarray(inputs["ln_g"], np.float32)
    lnb = np.asarray(inputs["ln_b"], np.float32)
    w2 = np.asarray(inputs["enc_w2"], np.float32)
    b2 = np.asarray(inputs["enc_b2"], np.float32)
    dw1 = np.ascontiguousarray(np.asarray(inputs["dec_w1"], np.float32))
    db1 = np.asarray(inputs["dec_b1"], np.float32)
    dw2 = np.asarray(inputs["dec_w2"], np.float32)
    db2 = np.asarray(inputs["dec_b2"], np.float32)

    assert np.all(lnb == 0.0) and np.all(lng > 0.0), "kernel assumes ln_b==0, ln_g>0"
    assert not np.any(b1) and not np.any(b2) and not np.any(db1) and not np.any(db2), \
        "kernel assumes zero biases"
    w2g = w2 * lng[:, None]

    e2sum = (cb.astype(np.float64) ** 2).sum(-1).astype(np.float32)  # [HQ, VOCAB]

    # sample rows: estimate per-level resid ranges + decoder ranges
    rng = np.random.default_rng(0)
    sel = rng.choice(x.shape[0], 1024, replace=False)
    h = x[sel] @ w1
    mu = h.mean(-1, keepdims=True)
    var = ((h - mu) ** 2).mean(-1, keepdims=True)
    hz = np.maximum((h - mu) / np.sqrt(var + LN_EPS) * lng, 0.0)
    lat_s = hz @ w2
    resid = lat_s.copy()
    rmaxs = []
    for lv in range(HQ):
        rmaxs.append(float(np.abs(resid).max()) * 1.3)
        sc = 2.0 * resid @ cb[lv].T - e2sum[lv]
        idx = sc.argmax(-1)
        resid = resid - cb[lv][idx]
    quant_s = lat_s - resid
    qmax = float(np.abs(quant_s).max()) * 1.3
    dh_s = np.maximum(quant_s @ dw1, 0.0)
    dhmax = float(np.abs(dh_s).max()) * 1.3

    krs = tuple(128.0 / r for r in rmaxs)
    kq = 128.0 / qmax
    S_d1 = 64.0
    S_dh = 128.0 / dhmax
    S_d2 = 1024.0
    krec = float(S_dh * S_d2)
    drelu = float(S_dh / (kq * S_d1))

    # per-level: |E|^2-sorted permutation, fp8 codebooks, group bias
    e2e8 = np.zeros((HQ, 2, 128, VOCAB // 2), f8)
    e2o8 = np.zeros((HQ, 2, 128, VOCAB // 2), f8)
    e2tp_pack = np.zeros((HQ, 128, VOCAB), np.uint32)
    gbias = np.zeros((HQ, 4), np.float32)
    for lv in range(HQ):
        perm = np.argsort(e2sum[lv], kind="stable")
        cbp = cb[lv][perm]                               # [VOCAB, LAT]
        S_E = 208.0 / float(np.abs(cbp).max())
        Et8 = q8c((S_E * cbp).T)                         # [LAT, VOCAB] fp8
        for k in range(2):
            e2e8[lv, k] = Et8[k * 128:(k + 1) * 128, 0::2]
            e2o8[lv, k] = Et8[k * 128:(k + 1) * 128, 1::2]
        # group bias: -(K_r*S_E/2)*mean|E|^2 per sorted bucket, grid-quantized
        bias_full = -(krs[lv] * S_E / 2.0) * e2sum[lv][perm]
        gb = np.array([bias_full[g * 2048:(g + 1) * 2048].mean() for g in range(4)])
        gbias[lv] = (np.round(gb / GRID) * GRID).astype(np.float32)
        # gather table: exact (permuted) codebook, bf16-packed pairs
        Etg = cbp.T.astype(ml_dtypes.bfloat16)           # [LAT, VOCAB]
        pk0 = Etg[:128].view(np.uint16).astype(np.uint32)
        pk1 = Etg[128:].view(np.uint16).astype(np.uint32)
        e2tp_pack[lv] = pk0 | (pk1 << 16)

    common = {
        "w1x8": np.ascontiguousarray(q8c(512.0 * w1)),
        "w2b": np.ascontiguousarray(w2g.astype(ml_dtypes.bfloat16)),
        "dw1x8": np.ascontiguousarray(q8c(S_d1 * dw1)),
        "dw2x8": np.ascontiguousarray(q8c(S_d2 * dw2)),
        "e2e8": np.ascontiguousarray(e2e8),
        "e2o8": np.ascontiguousarray(e2o8),
        "e2tp": e2tp_pack,
        "giota": np.ascontiguousarray(
            np.tile(np.arange(4, dtype=np.float32), (128, RT))
        ),
        "gbias": np.ascontiguousarray(
            np.tile(gbias.reshape(1, HQ * 4), (128, 1))
        ),
    }
    in_maps = []
    for c in range(NCORES):
        m = dict(common)
        xs = x[c * NSH:(c + 1) * NSH]
        m["x8t"] = np.ascontiguousarray(q8c(16.0 * xs).T)
        m["xtd"] = np.ascontiguousarray(
            (krec * xs).T.astype(ml_dtypes.bfloat16)
        )
        in_maps.append(m)
    meta = dict(krs=krs, kq=kq, drelu=drelu, krec=krec)
    return in_maps, meta


def _combine(results, meta):
    rlv = rrec = 0.0
    for c in range(NCORES):
        o = np.asarray(results[c]["out"], np.float64)
        rlv += o[:, 0:32].sum()
        rrec += o[:, 32:64].sum()
    rrec /= meta["krec"] ** 2
    return np.float32(1.5 * rlv / (N * LAT) + 0.5 * rrec / (N * OBS))


_NC_CACHE = {}


def get_nc(meta):
    key = (tuple(round(k, 5) for k in meta["krs"]),
           round(meta["kq"], 5), round(meta["drelu"], 7))
    if key not in _NC_CACHE:
        _NC_CACHE[key] = build_nc(meta["krs"], meta["kq"], meta["drelu"])
    return _NC_CACHE[key]


def kernel(**inputs) -> np.ndarray:
    in_maps, meta = _host_prep(inputs)
    nc = get_nc(meta)
    res = run_bass_kernel_spmd(nc, in_maps, core_ids=list(range(NCORES)))
    return _combine(res.results, meta)
